# revision 31
# baseline (speedup 1.0000x reference)
"""MoE layer (routed top-2 experts + shared SwiGLU expert) on 8 TRN2 NeuronCores.

Sharding strategy (per spec hint):
  - Routed experts: expert-parallel. Core e holds W1/W2/W3[e]; the host computes
    the router (bit-matching the reference's jax fp32 computation on CPU), gathers
    each expert's assigned tokens (top-2 of 8 per token => ~T/4 tokens per expert),
    and ships a [C, D] token block per core (C = max expert count). This is exact
    vs. the dense reference since w_full is zero for non-selected experts.
  - Shared expert: data-parallel on tokens. Core e processes tokens
    [e*T/8, (e+1)*T/8) through the full shared SwiGLU (weights replicated).
  - Combine: host scatter-add of weighted routed outputs + shared outputs.

Device kernel per core: two SwiGLU FFN passes (routed block, shared block):
    hT = (W1^T x^T) [H, Ttok]  (PSUM f32, accumulated over D/128 chunks)
    h2T = hT * silu(h3T)       (ACT sigmoid + DVE muls, cast to bf16)
    y  = h2T^T @ W2            [Tpad, D]  (bf16 out, 128-row-padded tiles)
All matmuls in bf16 with fp32 PSUM accumulation; y emitted in bf16 (the host
combine upcasts to f32 — quantization adds ~1e-4 rel, well inside the 2e-2
budget).
"""

from contextlib import ExitStack

import numpy as np
import ml_dtypes

import concourse.bacc as bacc
import concourse.tile as tile
from concourse import mybir
from concourse.bass_utils import run_bass_kernel_spmd

# Problem constants (hardcoded per the self-contained-kernel contract)
B, S, D, H, E, TOPK = 2, 2048, 1024, 2048, 8, 2
SCALE = 1.0 / float(np.sqrt(D))
NCORES = 8
P = 128
BF16 = ml_dtypes.bfloat16

# test.py introspection: last BassKernelResults (exec_time_ns when BASS_TRACE=1)
LAST_RESULTS = None

_NC_CACHE = {}

# Sigmoid+DVE-mul beats the ACT Silu table by ~54us on HW (cold-table cost),
# and CoreSim has no Silu — so the split path is the default everywhere.
SIM_COMPAT_SILU = True


def _ensure_ntff_hook():
    """run_bass_kernel_spmd(trace=True) imports antenv.axon_hooks, which this
    image's antenv lacks. Install a stub (wired to the ctypes NTFF profiler if
    available) so a BASS_TRACE=1 environment doesn't crash the kernel."""
    import sys
    import types

    try:
        import antenv.axon_hooks  # noqa: F401

        return
    except ImportError:
        pass
    try:
        import antenv
    except ImportError:
        return
    mod = types.ModuleType("antenv.axon_hooks")
    holder = [None]
    mod.set_axon_ntff_profile_hook = lambda h: holder.__setitem__(0, h)
    mod.get_axon_ntff_profile_hook = lambda: holder[0]
    sys.modules["antenv.axon_hooks"] = mod
    antenv.axon_hooks = mod
    try:
        import trn_agent_boot.trn_boot as tb

        mod.set_axon_ntff_profile_hook(
            tb._ntff_profile_via_ctypes("/opt/axon/libaxon_pjrt.so")
        )
    except Exception:
        pass
    # In hook-less images the artifact share upload is likely unavailable too;
    # make the trace path's upload best-effort instead of fatal.
    try:
        import concourse.bass_utils as bu

        _orig_upload = bu.upload_artifacts

        def _safe_upload(tmpdir):
            try:
                return _orig_upload(tmpdir)
            except Exception:
                return tmpdir

        bu.upload_artifacts = _safe_upload
    except Exception:
        pass


_ensure_ntff_hook()


def _token_chunks(t, step=512):
    """[(offset, size), ...] covering range(t) in chunks of <=step."""
    out = []
    o = 0
    while o < t:
        out.append((o, min(step, t - o)))
        o += step
    return out


def _balanced_chunks(t, step=512):
    """Like _token_chunks but near-equal sizes (avoids a thin tail chunk whose
    back-to-back LDWEIGHTS can't hide under very short matmuls)."""
    n = (t + step - 1) // step
    base, rem = divmod(t, n)
    out = []
    o = 0
    for i in range(n):
        s = base + (1 if i < rem else 0)
        out.append((o, s))
        o += s
    return out


def _emit_ffn(tc, pools, dram, Ttok, Tpad=None):
    """Emit one SwiGLU FFN pass: y[Tpad,D] = (x@W1 * silu(x@W3)) @ W2.

    dram: dict with xt [D,Ttok] bf16, w1/w3 [D,H] bf16, w2 [H,D] bf16,
          y [Tpad,D] bf16 DRAM APs. Tpad (>=Ttok, multiple of 128) pads
    phase B to full 128-row token tiles: partial-partition output DMAs
    drain on a single DMA engine (~16GB/s) and would sit on the kernel's
    critical path; rows Ttok..Tpad are zeroed garbage the host ignores.
    """
    nc = tc.nc
    if Tpad is None:
        Tpad = Ttok
    KD = D // P    # contraction chunks for phase A (8)
    MH = H // P    # h tiles (16)
    KH = H // P    # contraction chunks for phase B (16)
    # Balanced chunks avoid a thin tail chunk (512,512,67) whose LDWEIGHTS debt
    # showed up as per-mi bubbles; (364,364,363) streams at the N/2.4GHz floor.
    # Width capped at 448: at full clock a free-running ~10.8us periodic event
    # skips one issue slot of full-512-column matmuls (432ns each, ~13 per
    # kernel); 256-364-col streams are immune. Total columns are unchanged.
    chunks = _balanced_chunks(Ttok, 448)

    xt_d = dram["xt"].rearrange("(k p) t -> p k t", p=P)     # [128, KD, Ttok]
    w1_d = dram["w1"].rearrange("(k p) h -> p k h", p=P)     # [128, KD, H]
    w3_d = dram["w3"].rearrange("(k p) h -> p k h", p=P)
    w2_d = dram["w2"].rearrange("(k p) d -> k p d", p=P)     # [KH, 128, D]
    y_d = dram["y"]

    # Resident SBUF tensors (bufs=1 pools; pass 2 reuses the same slots)
    x_sb = pools["x"].tile([P, KD, Ttok], mybir.dt.bfloat16, tag="x_sb")
    w1_sb = pools["wA"].tile([P, KD, H], mybir.dt.bfloat16, tag="w1_sb")
    w3_sb = pools["wA"].tile([P, KD, H], mybir.dt.bfloat16, tag="w3_sb")
    w2_sb = pools["wB"].tile([P, KH, D], mybir.dt.bfloat16, tag="w2_sb")
    h2t_sb = pools["h2t"].tile([P, KH, Tpad], mybir.dt.bfloat16, tag="h2t_sb")
    if Tpad > Ttok:
        nc.vector.memset(h2t_sb[:, :, Ttok:Tpad], 0.0)

    # Batched DMAs in phase A's consumption order: one transfer for all of x
    # (needed before the first matmul), then w3/w1 interleaved in 256-column
    # panels (each panel feeds two mi iterations; w3 leads since ps3
    # accumulates first). Few, large DMAs: descriptor issue on the Sync queue
    # costs ~640ns each, which dominated startup with per-(k,tensor) DMAs.
    # (Slicing x into a narrow leading token range, and moving the panels to
    # GpSimd's DGE queue, were both tried and are not wins: sub-512B lines tank
    # packet efficiency, and the startup is bandwidth-bound, not issue-bound,
    # once the DMAs are batched.)
    nc.sync.dma_start(out=x_sb[:, :, :], in_=xt_d)
    WG = 256
    for c in range(0, H, WG):
        nc.sync.dma_start(out=w3_sb[:, :, c : c + WG], in_=w3_d[:, :, c : c + WG])
        nc.sync.dma_start(out=w1_sb[:, :, c : c + WG], in_=w1_d[:, :, c : c + WG])
    for k in range(KH):
        nc.sync.dma_start(out=w2_sb[:, k, :], in_=w2_d[k])

    # Phase A: h2T[H, Ttok] = (W1^T x^T) * silu(W3^T x^T), bf16.
    # k-outer per h-tile: each stationary weight chunk streams all token chunks
    # (fewer LDWEIGHTS, better hiding). All token-chunk PSUM tiles stay live.
    for mi in range(MH):
        hsl = slice(mi * P, (mi + 1) * P)
        for (o, nw) in chunks:
            # ps3 accumulates FIRST: its sigmoid+mul evict then overlaps ps1's
            # matmuls, leaving only the final h2t mul exposed after ps1 stops.
            ps3 = pools["psA"].tile([P, 512], mybir.dt.float32, tag="ps3", bufs=2)
            ps1 = pools["psA"].tile([P, 512], mybir.dt.float32, tag="ps1")
            for k in range(KD):
                nc.tensor.matmul(
                    ps3[:, :nw],
                    lhsT=w3_sb[:, k : k + 1, hsl],
                    rhs=x_sb[:, k : k + 1, o : o + nw],
                    start=(k == 0),
                    stop=(k == KD - 1),
                )
            for k in range(KD):
                nc.tensor.matmul(
                    ps1[:, :nw],
                    lhsT=w1_sb[:, k : k + 1, hsl],
                    rhs=x_sb[:, k : k + 1, o : o + nw],
                    start=(k == 0),
                    stop=(k == KD - 1),
                )
            # silu = h3 * sigmoid(h3). The split sigmoid+mul path is both
            # CoreSim-compatible and faster on HW than ACT's Silu table.
            sil = pools["tmp"].tile([P, 512], mybir.dt.float32, tag="sil")
            if SIM_COMPAT_SILU:
                sig = pools["tmp"].tile([P, 512], mybir.dt.float32, tag="sig")
                nc.scalar.activation(
                    sig[:, :nw], ps3[:, :nw], mybir.ActivationFunctionType.Sigmoid
                )
                nc.vector.tensor_mul(sil[:, :nw], ps3[:, :nw], sig[:, :nw])
            else:
                nc.scalar.activation(
                    sil[:, :nw], ps3[:, :nw], mybir.ActivationFunctionType.Silu
                )
            nc.vector.tensor_mul(h2t_sb[:, mi, o : o + nw], ps1[:, :nw], sil[:, :nw])

    # Phase B: y[Tpad, D] = h2T^T @ W2 (all token tiles full 128 rows)
    for (oj, njw) in _balanced_chunks(D, 448):  # D columns in chunks of ~342
        for ti in range(Tpad // P):
            tsl = slice(ti * P, (ti + 1) * P)
            ps = pools["psB"].tile([P, 512], mybir.dt.float32, tag="psB")
            for k in range(KH):
                nc.tensor.matmul(
                    ps[:, :njw],
                    lhsT=h2t_sb[:, k : k + 1, tsl],
                    rhs=w2_sb[:, k : k + 1, oj : oj + njw],
                    start=(k == 0),
                    stop=(k == KH - 1),
                )
            yst = pools["tmp"].tile([P, 512], mybir.dt.bfloat16, tag="yst")
            nc.vector.tensor_copy(out=yst[:, :njw], in_=ps[:, :njw])
            # Output DMAs ride the Scalar engine's HW DGE queue so they don't
            # queue behind (and stall) the input weight stream on Sync's queue.
            nc.scalar.dma_start(out=y_d[tsl, oj : oj + njw], in_=yst[:, :njw])


WARMUP_MMS = 20


def _build_nc(C, SS):
    """Build the per-core Bass program: routed FFN ([C] tokens) + shared FFN ([SS])."""
    nc = bacc.Bacc("TRN2", target_bir_lowering=False, debug=False)

    CP = ((C + P - 1) // P) * P  # phase-B-padded routed token count
    bf = mybir.dt.bfloat16
    routed = {
        "xt": nc.dram_tensor("xgt", [D, C], bf, kind="ExternalInput").ap(),
        "w1": nc.dram_tensor("w1", [D, H], bf, kind="ExternalInput").ap(),
        "w3": nc.dram_tensor("w3", [D, H], bf, kind="ExternalInput").ap(),
        "w2": nc.dram_tensor("w2", [H, D], bf, kind="ExternalInput").ap(),
        "y": nc.dram_tensor("yg", [CP, D], bf, kind="ExternalOutput").ap(),
    }
    shared = {
        "xt": nc.dram_tensor("xst", [D, SS], bf, kind="ExternalInput").ap(),
        "w1": nc.dram_tensor("ws1", [D, H], bf, kind="ExternalInput").ap(),
        "w3": nc.dram_tensor("ws3", [D, H], bf, kind="ExternalInput").ap(),
        "w2": nc.dram_tensor("ws2", [H, D], bf, kind="ExternalInput").ap(),
        "y": nc.dram_tensor("ys", [SS, D], bf, kind="ExternalOutput").ap(),
    }

    with tile.TileContext(nc) as tc, ExitStack() as ctx:
        pools = {
            "x": ctx.enter_context(tc.tile_pool(name="x", bufs=1)),
            "wA": ctx.enter_context(tc.tile_pool(name="wA", bufs=1)),
            "wB": ctx.enter_context(tc.tile_pool(name="wB", bufs=1)),
            "h2t": ctx.enter_context(tc.tile_pool(name="h2t", bufs=1)),
            "tmp": ctx.enter_context(tc.tile_pool(name="tmp", bufs=4)),
            "psA": ctx.enter_context(tc.tile_pool(name="psA", bufs=3, space="PSUM")),
            "psB": ctx.enter_context(tc.tile_pool(name="psB", bufs=3, space="PSUM")),
        }
        # HAM warm-up: dummy matmuls on a zeroed tile while the input DMAs
        # stream in, so the PE clock-gate is at 8/8 when real work starts.
        # N=512 wide so few instructions cover the ~6us until x+lead weights
        # land (cold MMs run at 1.2GHz: ~427ns each until HAM fires).
        warm = pools["tmp"].tile([P, 512], mybir.dt.bfloat16, tag="warm")
        nc.vector.memset(warm[:], 0.0)
        wps = pools["psA"].tile([P, 512], mybir.dt.float32, tag="ps1", name="wps")
        for i in range(WARMUP_MMS):
            nc.tensor.matmul(
                wps[:], lhsT=warm[:, :P], rhs=warm[:], start=True, stop=True
            )
        _emit_ffn(tc, pools, shared, SS)
        _emit_ffn(tc, pools, routed, C, Tpad=CP)

    nc.compile()
    return nc


def _route(x, Wr, rb):
    """Replicate the reference router. Returns (idx [T,2] int, w [T,2] f32).

    Uses jax on CPU with the exact expressions from the reference so the top-2
    selection bit-matches a CPU-run reference (min 2nd-vs-3rd logit gap in this
    problem is ~1e-6, so the selection must match the reference's fp32 math).
    Falls back to numpy float64 if jax-cpu is unavailable.
    """
    try:
        import jax
        import jax.numpy as jnp

        cpu = jax.devices("cpu")[0]
        with jax.default_device(cpu):
            xl = jnp.asarray(np.asarray(x))
            wr = jnp.asarray(np.asarray(Wr))
            rbj = jnp.asarray(np.asarray(rb))
            logits = jnp.einsum("bsd,de->bse", xl, wr) * SCALE
            _, idx = jax.lax.top_k(logits + rbj, TOPK)
            gathered = jnp.take_along_axis(logits, idx, axis=-1)
            w = jax.nn.softmax(gathered, axis=-1)
        idx = np.asarray(idx).reshape(-1, TOPK)
        w = np.asarray(w, dtype=np.float32).reshape(-1, TOPK)
        return idx, w
    except Exception:
        xf = np.asarray(x, np.float64).reshape(-1, D)
        logits = (xf @ np.asarray(Wr, np.float64)) * SCALE
        biased = logits + np.asarray(rb, np.float64)
        idx = np.argsort(-biased, axis=-1)[:, :TOPK]
        g = np.take_along_axis(logits, idx, axis=-1)
        g = g - g.max(axis=-1, keepdims=True)
        wexp = np.exp(g)
        w = (wexp / wexp.sum(axis=-1, keepdims=True)).astype(np.float32)
        return idx, w


def kernel(x, Wr, rb, W1, W2, W3, Ws1, Ws2, Ws3):
    global LAST_RESULTS
    x = np.asarray(x, np.float32)
    Wr = np.asarray(Wr, np.float32)
    rb = np.asarray(rb, np.float32)
    W1 = np.asarray(W1, np.float32)
    W2 = np.asarray(W2, np.float32)
    W3 = np.asarray(W3, np.float32)
    Ws1 = np.asarray(Ws1, np.float32)
    Ws2 = np.asarray(Ws2, np.float32)
    Ws3 = np.asarray(Ws3, np.float32)

    T = B * S
    xf = x.reshape(T, D)

    # ---- Router (host, exact) ----
    idx, w = _route(x, Wr, rb)

    # ---- Shard ----
    toks = [np.nonzero((idx == e).any(axis=1))[0] for e in range(E)]
    wtok = [
        w[toks[e], :][idx[toks[e], :] == e].astype(np.float32) for e in range(E)
    ]
    counts = [len(t) for t in toks]
    C = max(256, max(counts))  # exact max count; matmul free dims need no alignment
    SS = T // NCORES

    xf_bf = xf.astype(BF16)
    in_maps = []
    for e in range(E):
        xg = np.zeros((C, D), dtype=BF16)
        xg[: counts[e]] = xf_bf[toks[e]]
        in_maps.append(
            {
                "xgt": np.ascontiguousarray(xg.T),
                "w1": np.ascontiguousarray(W1[e].astype(BF16)),
                "w3": np.ascontiguousarray(W3[e].astype(BF16)),
                "w2": np.ascontiguousarray(W2[e].astype(BF16)),
                "xst": np.ascontiguousarray(xf_bf[e * SS : (e + 1) * SS].T),
                "ws1": np.ascontiguousarray(Ws1.astype(BF16)),
                "ws3": np.ascontiguousarray(Ws3.astype(BF16)),
                "ws2": np.ascontiguousarray(Ws2.astype(BF16)),
            }
        )

    # ---- Device ----
    key = (C, SS)
    if key not in _NC_CACHE:
        _NC_CACHE[key] = _build_nc(C, SS)
    nc = _NC_CACHE[key]
    res = run_bass_kernel_spmd(nc, in_maps, list(range(NCORES)))
    LAST_RESULTS = res

    # ---- Combine (host) ----
    out = np.empty((T, D), dtype=np.float32)
    for e in range(E):
        out[e * SS : (e + 1) * SS] = res.results[e]["ys"].astype(np.float32)
    for e in range(E):
        yg = res.results[e]["yg"][: counts[e]].astype(np.float32)
        out[toks[e]] += wtok[e][:, None] * yg
    return out.reshape(B, S, D)



# revision 33
# speedup vs baseline: 1.0030x; 1.0030x over previous
"""MoE layer (routed top-2 experts + shared SwiGLU expert) on 8 TRN2 NeuronCores.

Sharding strategy (per spec hint):
  - Routed experts: expert-parallel. Core e holds W1/W2/W3[e]; the host computes
    the router (bit-matching the reference's jax fp32 computation on CPU), gathers
    each expert's assigned tokens (top-2 of 8 per token => ~T/4 tokens per expert),
    and ships a [C, D] token block per core (C = max expert count). This is exact
    vs. the dense reference since w_full is zero for non-selected experts.
  - Shared expert: data-parallel on tokens. Core e processes tokens
    [e*T/8, (e+1)*T/8) through the full shared SwiGLU (weights replicated).
  - Combine: host scatter-add of weighted routed outputs + shared outputs.

Device kernel per core: two SwiGLU FFN passes (routed block, shared block):
    hT = (W1^T x^T) [H, Ttok]  (PSUM f32, accumulated over D/128 chunks)
    h2T = hT * silu(h3T)       (ACT sigmoid + DVE muls, cast to bf16)
    y  = h2T^T @ W2            [Tpad, D]  (bf16 out, 128-row-padded tiles)
All matmuls in bf16 with fp32 PSUM accumulation; y emitted in bf16 (the host
combine upcasts to f32 — quantization adds ~1e-4 rel, well inside the 2e-2
budget).
"""

from contextlib import ExitStack

import numpy as np
import ml_dtypes

import concourse.bacc as bacc
import concourse.tile as tile
from concourse import mybir
from concourse.bass_utils import run_bass_kernel_spmd

# Problem constants (hardcoded per the self-contained-kernel contract)
B, S, D, H, E, TOPK = 2, 2048, 1024, 2048, 8, 2
SCALE = 1.0 / float(np.sqrt(D))
NCORES = 8
P = 128
BF16 = ml_dtypes.bfloat16

# test.py introspection: last BassKernelResults (exec_time_ns when BASS_TRACE=1)
LAST_RESULTS = None

_NC_CACHE = {}

# Sigmoid+DVE-mul beats the ACT Silu table by ~54us on HW (cold-table cost),
# and CoreSim has no Silu — so the split path is the default everywhere.
SIM_COMPAT_SILU = True


def _ensure_ntff_hook():
    """run_bass_kernel_spmd(trace=True) imports antenv.axon_hooks, which this
    image's antenv lacks. Install a stub (wired to the ctypes NTFF profiler if
    available) so a BASS_TRACE=1 environment doesn't crash the kernel."""
    import sys
    import types

    try:
        import antenv.axon_hooks  # noqa: F401

        return
    except ImportError:
        pass
    try:
        import antenv
    except ImportError:
        return
    mod = types.ModuleType("antenv.axon_hooks")
    holder = [None]
    mod.set_axon_ntff_profile_hook = lambda h: holder.__setitem__(0, h)
    mod.get_axon_ntff_profile_hook = lambda: holder[0]
    sys.modules["antenv.axon_hooks"] = mod
    antenv.axon_hooks = mod
    try:
        import trn_agent_boot.trn_boot as tb

        mod.set_axon_ntff_profile_hook(
            tb._ntff_profile_via_ctypes("/opt/axon/libaxon_pjrt.so")
        )
    except Exception:
        pass
    # In hook-less images the artifact share upload is likely unavailable too;
    # make the trace path's upload best-effort instead of fatal.
    try:
        import concourse.bass_utils as bu

        _orig_upload = bu.upload_artifacts

        def _safe_upload(tmpdir):
            try:
                return _orig_upload(tmpdir)
            except Exception:
                return tmpdir

        bu.upload_artifacts = _safe_upload
    except Exception:
        pass


_ensure_ntff_hook()


def _token_chunks(t, step=512):
    """[(offset, size), ...] covering range(t) in chunks of <=step."""
    out = []
    o = 0
    while o < t:
        out.append((o, min(step, t - o)))
        o += step
    return out


def _balanced_chunks(t, step=512):
    """Like _token_chunks but near-equal sizes (avoids a thin tail chunk whose
    back-to-back LDWEIGHTS can't hide under very short matmuls)."""
    n = (t + step - 1) // step
    base, rem = divmod(t, n)
    out = []
    o = 0
    for i in range(n):
        s = base + (1 if i < rem else 0)
        out.append((o, s))
        o += s
    return out


def _emit_ffn(tc, pools, dram, Ttok, Tpad=None):
    """Emit one SwiGLU FFN pass: y[Tpad,D] = (x@W1 * silu(x@W3)) @ W2.

    dram: dict with xt [D,Ttok] bf16, w1/w3 [D,H] bf16, w2 [H,D] bf16,
          y [Tpad,D] bf16 DRAM APs. Tpad (>=Ttok, multiple of 128) pads
    phase B to full 128-row token tiles: partial-partition output DMAs
    drain on a single DMA engine (~16GB/s) and would sit on the kernel's
    critical path; rows Ttok..Tpad are zeroed garbage the host ignores.
    """
    nc = tc.nc
    if Tpad is None:
        Tpad = Ttok
    KD = D // P    # contraction chunks for phase A (8)
    MH = H // P    # h tiles (16)
    KH = H // P    # contraction chunks for phase B (16)
    # Balanced chunks avoid a thin tail chunk (512,512,67) whose LDWEIGHTS debt
    # showed up as per-mi bubbles; (364,364,363) streams at the N/2.4GHz floor.
    # (Capping width at 448 kills the ~10.8us-periodic slot-skips that hit
    # 512-col matmuls at full clock, but the extra PSUM-group boundaries cost
    # the same ~5us back — measured a wash; keep the coarser chunks.)
    chunks = _balanced_chunks(Ttok, 512)

    xt_d = dram["xt"].rearrange("(k p) t -> p k t", p=P)     # [128, KD, Ttok]
    w1_d = dram["w1"].rearrange("(k p) h -> p k h", p=P)     # [128, KD, H]
    w3_d = dram["w3"].rearrange("(k p) h -> p k h", p=P)
    w2_d = dram["w2"].rearrange("(k p) d -> k p d", p=P)     # [KH, 128, D]
    y_d = dram["y"]

    # Resident SBUF tensors (bufs=1 pools; pass 2 reuses the same slots)
    x_sb = pools["x"].tile([P, KD, Ttok], mybir.dt.bfloat16, tag="x_sb")
    w1_sb = pools["wA"].tile([P, KD, H], mybir.dt.bfloat16, tag="w1_sb")
    w3_sb = pools["wA"].tile([P, KD, H], mybir.dt.bfloat16, tag="w3_sb")
    w2_sb = pools["wB"].tile([P, KH, D], mybir.dt.bfloat16, tag="w2_sb")
    h2t_sb = pools["h2t"].tile([P, KH, Tpad], mybir.dt.bfloat16, tag="h2t_sb")
    if Tpad > Ttok:
        nc.vector.memset(h2t_sb[:, :, Ttok:Tpad], 0.0)

    # Batched DMAs in phase A's consumption order: one transfer for all of x
    # (needed before the first matmul), then w3/w1 interleaved in 256-column
    # panels (each panel feeds two mi iterations; w3 leads since ps3
    # accumulates first). Few, large DMAs: descriptor issue on the Sync queue
    # costs ~640ns each, which dominated startup with per-(k,tensor) DMAs.
    # (Slicing x into a narrow leading token range, and moving the panels to
    # GpSimd's DGE queue, were both tried and are not wins: sub-512B lines tank
    # packet efficiency, and the startup is bandwidth-bound, not issue-bound,
    # once the DMAs are batched.)
    nc.sync.dma_start(out=x_sb[:, :, :], in_=xt_d)
    WG = 256
    for c in range(0, H, WG):
        nc.sync.dma_start(out=w3_sb[:, :, c : c + WG], in_=w3_d[:, :, c : c + WG])
        nc.sync.dma_start(out=w1_sb[:, :, c : c + WG], in_=w1_d[:, :, c : c + WG])
    for k in range(KH):
        nc.sync.dma_start(out=w2_sb[:, k, :], in_=w2_d[k])

    # Phase A: h2T[H, Ttok] = (W1^T x^T) * silu(W3^T x^T), bf16.
    # k-outer per h-tile: each stationary weight chunk streams all token chunks
    # (fewer LDWEIGHTS, better hiding). All token-chunk PSUM tiles stay live.
    for mi in range(MH):
        hsl = slice(mi * P, (mi + 1) * P)
        for (o, nw) in chunks:
            # ps3 accumulates FIRST: its sigmoid+mul evict then overlaps ps1's
            # matmuls, leaving only the final h2t mul exposed after ps1 stops.
            ps3 = pools["psA"].tile([P, 512], mybir.dt.float32, tag="ps3", bufs=2)
            ps1 = pools["psA"].tile([P, 512], mybir.dt.float32, tag="ps1")
            for k in range(KD):
                nc.tensor.matmul(
                    ps3[:, :nw],
                    lhsT=w3_sb[:, k : k + 1, hsl],
                    rhs=x_sb[:, k : k + 1, o : o + nw],
                    start=(k == 0),
                    stop=(k == KD - 1),
                )
            for k in range(KD):
                nc.tensor.matmul(
                    ps1[:, :nw],
                    lhsT=w1_sb[:, k : k + 1, hsl],
                    rhs=x_sb[:, k : k + 1, o : o + nw],
                    start=(k == 0),
                    stop=(k == KD - 1),
                )
            # silu = h3 * sigmoid(h3). The split sigmoid+mul path is both
            # CoreSim-compatible and faster on HW than ACT's Silu table.
            sil = pools["tmp"].tile([P, 512], mybir.dt.float32, tag="sil")
            if SIM_COMPAT_SILU:
                sig = pools["tmp"].tile([P, 512], mybir.dt.float32, tag="sig")
                nc.scalar.activation(
                    sig[:, :nw], ps3[:, :nw], mybir.ActivationFunctionType.Sigmoid
                )
                nc.vector.tensor_mul(sil[:, :nw], ps3[:, :nw], sig[:, :nw])
            else:
                nc.scalar.activation(
                    sil[:, :nw], ps3[:, :nw], mybir.ActivationFunctionType.Silu
                )
            nc.vector.tensor_mul(h2t_sb[:, mi, o : o + nw], ps1[:, :nw], sil[:, :nw])

    # Phase B: y[Tpad, D] = h2T^T @ W2 (all token tiles full 128 rows)
    for (oj, njw) in _balanced_chunks(D, 512):  # D columns in chunks of 512
        for ti in range(Tpad // P):
            tsl = slice(ti * P, (ti + 1) * P)
            ps = pools["psB"].tile([P, 512], mybir.dt.float32, tag="psB")
            for k in range(KH):
                nc.tensor.matmul(
                    ps[:, :njw],
                    lhsT=h2t_sb[:, k : k + 1, tsl],
                    rhs=w2_sb[:, k : k + 1, oj : oj + njw],
                    start=(k == 0),
                    stop=(k == KH - 1),
                )
            yst = pools["tmp"].tile([P, 512], mybir.dt.bfloat16, tag="yst")
            nc.vector.tensor_copy(out=yst[:, :njw], in_=ps[:, :njw])
            # Output DMAs ride the Scalar engine's HW DGE queue so they don't
            # queue behind (and stall) the input weight stream on Sync's queue.
            nc.scalar.dma_start(out=y_d[tsl, oj : oj + njw], in_=yst[:, :njw])


WARMUP_MMS = 20


def _build_nc(C, SS):
    """Build the per-core Bass program: routed FFN ([C] tokens) + shared FFN ([SS])."""
    nc = bacc.Bacc("TRN2", target_bir_lowering=False, debug=False)

    CP = ((C + P - 1) // P) * P  # phase-B-padded routed token count
    bf = mybir.dt.bfloat16
    routed = {
        "xt": nc.dram_tensor("xgt", [D, C], bf, kind="ExternalInput").ap(),
        "w1": nc.dram_tensor("w1", [D, H], bf, kind="ExternalInput").ap(),
        "w3": nc.dram_tensor("w3", [D, H], bf, kind="ExternalInput").ap(),
        "w2": nc.dram_tensor("w2", [H, D], bf, kind="ExternalInput").ap(),
        "y": nc.dram_tensor("yg", [CP, D], bf, kind="ExternalOutput").ap(),
    }
    shared = {
        "xt": nc.dram_tensor("xst", [D, SS], bf, kind="ExternalInput").ap(),
        "w1": nc.dram_tensor("ws1", [D, H], bf, kind="ExternalInput").ap(),
        "w3": nc.dram_tensor("ws3", [D, H], bf, kind="ExternalInput").ap(),
        "w2": nc.dram_tensor("ws2", [H, D], bf, kind="ExternalInput").ap(),
        "y": nc.dram_tensor("ys", [SS, D], bf, kind="ExternalOutput").ap(),
    }

    with tile.TileContext(nc) as tc, ExitStack() as ctx:
        pools = {
            "x": ctx.enter_context(tc.tile_pool(name="x", bufs=1)),
            "wA": ctx.enter_context(tc.tile_pool(name="wA", bufs=1)),
            "wB": ctx.enter_context(tc.tile_pool(name="wB", bufs=1)),
            "h2t": ctx.enter_context(tc.tile_pool(name="h2t", bufs=1)),
            "tmp": ctx.enter_context(tc.tile_pool(name="tmp", bufs=4)),
            "psA": ctx.enter_context(tc.tile_pool(name="psA", bufs=3, space="PSUM")),
            "psB": ctx.enter_context(tc.tile_pool(name="psB", bufs=3, space="PSUM")),
        }
        # HAM warm-up: dummy matmuls on a zeroed tile while the input DMAs
        # stream in, so the PE clock-gate is at 8/8 when real work starts.
        # N=512 wide so few instructions cover the ~6us until x+lead weights
        # land (cold MMs run at 1.2GHz: ~427ns each until HAM fires).
        warm = pools["tmp"].tile([P, 512], mybir.dt.bfloat16, tag="warm")
        nc.vector.memset(warm[:], 0.0)
        wps = pools["psA"].tile([P, 512], mybir.dt.float32, tag="ps1", name="wps")
        for i in range(WARMUP_MMS):
            nc.tensor.matmul(
                wps[:], lhsT=warm[:, :P], rhs=warm[:], start=True, stop=True
            )
        _emit_ffn(tc, pools, shared, SS)
        _emit_ffn(tc, pools, routed, C, Tpad=CP)

    nc.compile()
    return nc


def _route(x, Wr, rb):
    """Replicate the reference router. Returns (idx [T,2] int, w [T,2] f32).

    Uses jax on CPU with the exact expressions from the reference so the top-2
    selection bit-matches a CPU-run reference (min 2nd-vs-3rd logit gap in this
    problem is ~1e-6, so the selection must match the reference's fp32 math).
    Falls back to numpy float64 if jax-cpu is unavailable.
    """
    try:
        import jax
        import jax.numpy as jnp

        cpu = jax.devices("cpu")[0]
        with jax.default_device(cpu):
            xl = jnp.asarray(np.asarray(x))
            wr = jnp.asarray(np.asarray(Wr))
            rbj = jnp.asarray(np.asarray(rb))
            logits = jnp.einsum("bsd,de->bse", xl, wr) * SCALE
            _, idx = jax.lax.top_k(logits + rbj, TOPK)
            gathered = jnp.take_along_axis(logits, idx, axis=-1)
            w = jax.nn.softmax(gathered, axis=-1)
        idx = np.asarray(idx).reshape(-1, TOPK)
        w = np.asarray(w, dtype=np.float32).reshape(-1, TOPK)
        return idx, w
    except Exception:
        xf = np.asarray(x, np.float64).reshape(-1, D)
        logits = (xf @ np.asarray(Wr, np.float64)) * SCALE
        biased = logits + np.asarray(rb, np.float64)
        idx = np.argsort(-biased, axis=-1)[:, :TOPK]
        g = np.take_along_axis(logits, idx, axis=-1)
        g = g - g.max(axis=-1, keepdims=True)
        wexp = np.exp(g)
        w = (wexp / wexp.sum(axis=-1, keepdims=True)).astype(np.float32)
        return idx, w


def kernel(x, Wr, rb, W1, W2, W3, Ws1, Ws2, Ws3):
    global LAST_RESULTS
    x = np.asarray(x, np.float32)
    Wr = np.asarray(Wr, np.float32)
    rb = np.asarray(rb, np.float32)
    W1 = np.asarray(W1, np.float32)
    W2 = np.asarray(W2, np.float32)
    W3 = np.asarray(W3, np.float32)
    Ws1 = np.asarray(Ws1, np.float32)
    Ws2 = np.asarray(Ws2, np.float32)
    Ws3 = np.asarray(Ws3, np.float32)

    T = B * S
    xf = x.reshape(T, D)

    # ---- Router (host, exact) ----
    idx, w = _route(x, Wr, rb)

    # ---- Shard ----
    toks = [np.nonzero((idx == e).any(axis=1))[0] for e in range(E)]
    wtok = [
        w[toks[e], :][idx[toks[e], :] == e].astype(np.float32) for e in range(E)
    ]
    counts = [len(t) for t in toks]
    C = max(256, max(counts))  # exact max count; matmul free dims need no alignment
    SS = T // NCORES

    xf_bf = xf.astype(BF16)
    in_maps = []
    for e in range(E):
        xg = np.zeros((C, D), dtype=BF16)
        xg[: counts[e]] = xf_bf[toks[e]]
        in_maps.append(
            {
                "xgt": np.ascontiguousarray(xg.T),
                "w1": np.ascontiguousarray(W1[e].astype(BF16)),
                "w3": np.ascontiguousarray(W3[e].astype(BF16)),
                "w2": np.ascontiguousarray(W2[e].astype(BF16)),
                "xst": np.ascontiguousarray(xf_bf[e * SS : (e + 1) * SS].T),
                "ws1": np.ascontiguousarray(Ws1.astype(BF16)),
                "ws3": np.ascontiguousarray(Ws3.astype(BF16)),
                "ws2": np.ascontiguousarray(Ws2.astype(BF16)),
            }
        )

    # ---- Device ----
    key = (C, SS)
    if key not in _NC_CACHE:
        _NC_CACHE[key] = _build_nc(C, SS)
    nc = _NC_CACHE[key]
    res = run_bass_kernel_spmd(nc, in_maps, list(range(NCORES)))
    LAST_RESULTS = res

    # ---- Combine (host) ----
    out = np.empty((T, D), dtype=np.float32)
    for e in range(E):
        out[e * SS : (e + 1) * SS] = res.results[e]["ys"].astype(np.float32)
    for e in range(E):
        yg = res.results[e]["yg"][: counts[e]].astype(np.float32)
        out[toks[e]] += wtok[e][:, None] * yg
    return out.reshape(B, S, D)

